# revision 22
# baseline (speedup 1.0000x reference)
"""Tensor-parallel fused attention block (QKV proj + MHA + out proj) for 8 TRN2 cores.

Sharding: 16 heads -> 2 heads per core. W1 rows (q/k/v of the core's heads) and
W2 columns are sharded; x is replicated. Each core computes a partial output
[B*T, E] (its heads' contribution through W2); the host sums the 8 partials.

v3: single software-pipelined PE stream. The attention inner loop (ST -> exp ->
PV, with PV emitted two kt behind ST so the ACT exp latency is always hidden)
is the backbone; QKV matmul groups of the NEXT batch and out-projection groups
of finished chunks are interleaved into it as cost-budgeted filler so the
in-order PE queue never idles while ACT paces exp. Softmax denominators per
(head, chunk): a DVE pairwise tree sums the 16 exp tiles (rowsum matmuls
measured serial on HW - 16 matmuls/unit cost 169 us/iter in the v2 design),
then ONE ones_col[128,128] bf16 matmul both reduces the summed tile over
partitions and broadcasts the denominator, then DVE reciprocal + multiply.
That norm chain is deferred into the NEXT unit's early kt slots so the DVE
tail never stalls PE; it is emitted inline (never queued behind filler, which
would deadlock single-buffered PSUM pools against the in-order engine queues).
Batch boundaries force-drain only the next batch's QKV items (correctness:
their writes must be emitted before the attention reads / pool-slot reuse);
leftover projection filler flows into the next batch.

Layouts (host-prepped, all bf16):
  xT   [E, B*T]   x transposed, feature-major (rhs/lhsT tiles for QKV matmuls)
  wqT  [E, 256]   w1-q rows for the core's 2 heads, transposed
  wkT  [E, 256]
  wvT  [E, 256]
  w2sT [256, E]   w2 columns for the core's heads, transposed
"""

import collections
import json
import types

import ml_dtypes
import numpy as np

B, T, E = 4, 2048, 2048
NH, D = 16, 128
NCORES = 8
HPC = NH // NCORES          # heads per core = 2
F = HPC * D                 # per-core qkv feature dim = 256
N = B * T                   # 8192 tokens
ET = E // 128               # 16 e-tiles
SCALE = float(1.0 / np.sqrt(D))

BF16 = ml_dtypes.bfloat16


def _split_multi_waits(m: dict) -> dict:
    """This container's walrus rejects any instruction carrying >1 semaphore
    wait; hoist extra waits into standalone single-wait EventSemaphore insts
    emitted just before, on the same engine (program order preserves semantics)."""
    for fn in m["functions"]:
        for b in fn["blocks"]:
            new_insts = []
            for i in b["instructions"]:
                si = i.get("sync_info")
                waits = (si or {}).get("on_wait") or []
                if len(waits) > 1:
                    for k, w in enumerate(waits[:-1]):
                        new_insts.append({
                            "name": f"{i['name']}-presplitwait-{k}",
                            "opcode": "EventSemaphore",
                            "engine": i["engine"],
                            "ins": [], "outs": [],
                            "sync_info": {"on_wait": [w], "on_update": []},
                        })
                    si["on_wait"] = [waits[-1]]
                new_insts.append(i)
            b["instructions"] = new_insts
    return m


def _patch_serializer(nc):
    orig = nc.to_json_bytes

    def to_json_bytes(self):
        return json.dumps(_split_multi_waits(json.loads(orig()))).encode()

    nc.to_json_bytes = types.MethodType(to_json_bytes, nc)


def build_nc(loop: int | None = None, ablate: frozenset | set = frozenset()):
    import contextlib

    import concourse.bass as bass
    import concourse.mybir as mybir
    import concourse.tile as tile

    dt = mybir.dt
    AF = mybir.ActivationFunctionType

    nc = bass.Bass("TRN2", target_bir_lowering=False, debug=False)

    xT = nc.dram_tensor("xT", [N // 512, 128, ET, 512], dt.bfloat16,
                        kind="ExternalInput")
    wqT = nc.dram_tensor("wqT", [E, F], dt.bfloat16, kind="ExternalInput")
    wkT = nc.dram_tensor("wkT", [E, F], dt.bfloat16, kind="ExternalInput")
    wvT = nc.dram_tensor("wvT", [E, F], dt.bfloat16, kind="ExternalInput")
    w2sT = nc.dram_tensor("w2sT", [F, E], dt.bfloat16, kind="ExternalInput")
    out = nc.dram_tensor("out", [N, E], dt.bfloat16, kind="ExternalOutput")

    wq_r = wqT.rearrange("(n p) f -> p n f", p=128)     # [128, 16, 256]
    wk_r = wkT.rearrange("(n p) f -> p n f", p=128)
    wv_r = wvT.rearrange("(n p) f -> p n f", p=128)
    w2_r = w2sT.rearrange("(n p) f -> p n f", p=128)    # [128, 2, 2048]

    with tile.TileContext(nc) as tc:
        with (
            tc.tile_pool(name="wpool", bufs=1) as wpool,
            tc.tile_pool(name="xpool", bufs=4) as xpool,
            tc.tile_pool(name="qkv", bufs=2) as qkvpool,
            tc.tile_pool(name="ptpool", bufs=8) as ptpool,
            tc.tile_pool(name="otpool", bufs=2) as otpool,
            tc.tile_pool(name="opool", bufs=4) as opool,
            tc.tile_pool(name="small", bufs=3) as small,
            tc.tile_pool(name="sums", bufs=8) as sums,
            tc.tile_pool(name="ps_mm", bufs=2, space="PSUM") as ps_mm,
            tc.tile_pool(name="ps_st", bufs=3, space="PSUM") as ps_st,
            tc.tile_pool(name="ps_ot", bufs=2, space="PSUM") as ps_ot,
            tc.tile_pool(name="ps_bc", bufs=1, space="PSUM") as ps_bc,
        ):
            # weights + constants (loaded once)
            wq_sb = wpool.tile([128, ET, F], dt.bfloat16)
            wk_sb = wpool.tile([128, ET, F], dt.bfloat16)
            wv_sb = wpool.tile([128, ET, F], dt.bfloat16)
            w2_sb = wpool.tile([128, 2, E], dt.bfloat16)
            # wq first: the opening q-projection groups wait only on wq +
            # x chunk 0, so those head every DMA queue; wk/wv/w2 follow
            for g in range(4):
                sl = slice(g * 4, (g + 1) * 4)
                nc.sync.dma_start(out=wq_sb[:, sl, :], in_=wq_r[:, sl, :])
            for g in range(4):
                sl = slice(g * 4, (g + 1) * 4)
                nc.sync.dma_start(out=wk_sb[:, sl, :], in_=wk_r[:, sl, :])
            for g in range(4):
                sl = slice(g * 4, (g + 1) * 4)
                nc.sync.dma_start(out=wv_sb[:, sl, :], in_=wv_r[:, sl, :])
                nc.sync.dma_start(
                    out=w2_sb[:, :, g * 512:(g + 1) * 512],
                    in_=w2_r[:, :, g * 512:(g + 1) * 512])
            # all-ones [128,128]: one bf16 matmul both reduces the partition
            # dim and broadcasts the total to every output partition
            ones_col = wpool.tile([128, 128], dt.bfloat16)
            nc.vector.memset(ones_col, 1.0)

            loop_cm = tc.For_i(0, loop, 1) if loop else contextlib.nullcontext()
            with loop_cm:
                _emit_body(nc, tc, dt, AF, locals(), ablate)
    _patch_serializer(nc)
    return nc


def _emit_body(nc, tc, dt, AF, env, ablate=frozenset()):
    xT_r, out = env["xT"], env["out"]
    wq_sb, wk_sb, wv_sb, w2_sb = env["wq_sb"], env["wk_sb"], env["wv_sb"], env["w2_sb"]
    ones_col = env["ones_col"]
    xpool, qkvpool, ptpool, otpool, opool, small, sums = (
        env["xpool"], env["qkvpool"], env["ptpool"], env["otpool"],
        env["opool"], env["small"], env["sums"])
    ps_mm, ps_st, ps_ot, ps_bc = (
        env["ps_mm"], env["ps_st"], env["ps_ot"], env["ps_bc"])

    # filler queue of (pe_cost_ns, emit_fn); drained on a cost budget so the
    # chunky items spread evenly across the attention kt slots
    fillq = collections.deque()
    debt = [0.0]

    def drain(budget):
        debt[0] += budget
        while debt[0] > 0 and fillq:
            cost, fn = fillq.popleft()
            fn()
            debt[0] -= cost

    def drain_all():
        while fillq:
            fillq.popleft()[1]()
        debt[0] = 0.0

    qkv_tiles = {}

    def push_qkv_items(b):
        """Queue batch b's QKV projection as whole-group filler items (each
        item allocates, fills, and evacuates its own PSUM bank - holding a
        pool slot across items would deadlock the in-order PE queue)."""
        qkv_tiles[b] = (
            qkvpool.tile([128, HPC, T], dt.bfloat16, tag="qT", name="qT_sb"),
            qkvpool.tile([128, HPC, T], dt.bfloat16, tag="kT", name="kT_sb"),
            qkvpool.tile([128, T // 128, F], dt.bfloat16, tag="v", name="v_sb"),
        )
        qT_sb, kT_sb, v_sb = qkv_tiles[b]
        for c in range(T // 512):  # 4 chunks of 512 tokens
            x_tile = xpool.tile([128, ET, 512], dt.bfloat16, tag="x")
            ci = b * 4 + c

            def dma_x(x_tile=x_tile, ci=ci):
                for g in range(4):
                    nc.sync.dma_start(
                        out=x_tile[:, g * 4:(g + 1) * 4, :],
                        in_=xT_r[ci, :, g * 4:(g + 1) * 4, :])
            fillq.append((0.0, dma_x))

            for wi, (w_sb, dst) in enumerate(
                    ((wq_sb, qT_sb), (wk_sb, kT_sb))):
                for h in range(HPC):
                    def qk_grp(w_sb=w_sb, dst=dst, x_tile=x_tile, h=h, c=c,
                               wi=wi):
                        ps = ps_mm.tile([128, 512], dt.float32, tag="mm")
                        for e in range(ET):
                            nc.tensor.matmul(
                                ps, lhsT=w_sb[:, e, h * 128:(h + 1) * 128],
                                rhs=x_tile[:, e, :],
                                start=(e == 0), stop=(e == ET - 1))
                        cp = (nc.vector.tensor_copy if wi == 0
                              else nc.scalar.copy)
                        cp(out=dst[:, h, c * 512:(c + 1) * 512], in_=ps)
                    fillq.append((3413.0, qk_grp))

            for nn in range(4):  # token tiles of 128 within chunk
                def v_grp(x_tile=x_tile, nn=nn, c=c, v_sb=v_sb):
                    ps = ps_mm.tile([128, F], dt.float32, tag="mm")
                    for e in range(ET):
                        nc.tensor.matmul(
                            ps, lhsT=x_tile[:, e, nn * 128:(nn + 1) * 128],
                            rhs=wv_sb[:, e, :],
                            start=(e == 0), stop=(e == ET - 1))
                    nc.scalar.copy(out=v_sb[:, c * 4 + nn, :], in_=ps)
                fillq.append((1707.0, v_grp))

    def push_proj_items(b, c, ot_sb):
        """Out-projection for token chunk c of batch b (both heads normalized)."""
        for nn in range(c * 4, (c + 1) * 4):  # 4 token tiles of 128
            o_sb = opool.tile([128, E], dt.bfloat16, tag="o")
            for oc in range(E // 512):
                def p_grp(ot_sb=ot_sb, o_sb=o_sb, nn=nn, oc=oc):
                    ps = ps_mm.tile([128, 512], dt.float32, tag="mm")
                    for j in range(HPC):
                        nc.tensor.matmul(
                            ps, lhsT=ot_sb[:, j, nn * 128:(nn + 1) * 128],
                            rhs=w2_sb[:, j, oc * 512:(oc + 1) * 512],
                            start=(j == 0), stop=(j == HPC - 1))
                    cp = (nc.vector.tensor_copy if oc % 2 == 0
                          else nc.scalar.copy)
                    cp(out=o_sb[:, oc * 512:(oc + 1) * 512], in_=ps)
                fillq.append((480.0, p_grp))

            def p_dma(o_sb=o_sb, b=b, nn=nn):
                nc.sync.dma_start(
                    out=out[b * T + nn * 128:b * T + (nn + 1) * 128, :],
                    in_=o_sb)
            fillq.append((0.0, p_dma))

    # ---- batch 0 QKV: emitted dense upfront ----
    push_qkv_items(0)
    drain_all()

    def emit_norm(pn):
        """Deferred normalization for a finished unit: one ones_col matmul
        reduces the summed-P tile over partitions AND broadcasts the
        denominator; then DVE reciprocal + multiply. Emitted during the NEXT
        unit's early kt slots so the DVE tree-add tail never stalls PE."""
        total, ot_ps, dst, bb, cc, hh, ot_sb_ = pn
        bc = ps_bc.tile([128, 512], dt.float32, tag="bc")
        nc.tensor.matmul(bc, lhsT=ones_col, rhs=total, start=True, stop=True)
        recip = small.tile([128, 512], dt.float32, tag="recip")
        nc.vector.reciprocal(out=recip, in_=bc)
        nc.vector.tensor_mul(out=dst, in0=ot_ps, in1=recip)
        if hh == 1:
            push_proj_items(bb, cc, ot_sb_)

    # DVE pairwise tree over the 16 exp tiles: emit each add at the kt slot
    # where both operands' exps are >= 2 slots old (DVE runs them in order)
    TREE_AT = {3: [("s0", 0, 1)], 5: [("s1", 2, 3)], 6: [("m0", "s0", "s1")],
               7: [("s2", 4, 5)], 9: [("s3", 6, 7)], 10: [("m1", "s2", "s3")],
               11: [("s4", 8, 9), ("q0", "m0", "m1")], 13: [("s5", 10, 11)],
               14: [("m2", "s4", "s5")], 15: [("s6", 12, 13)]}
    TREE_TAIL = [("s7", 14, 15), ("m3", "s6", "s7"), ("q1", "m2", "m3"),
                 ("tot", "q0", "q1")]

    qkv_done = {}
    for b in range(B):
        # correctness barrier: batch b's attention reads qkv(b) tiles, whose
        # writer items must be EMITTED before the first ST reads them - and
        # before push_qkv_items(b+1) reuses their x-tile pool slots
        while b in qkv_done and not qkv_done[b]:
            drain(100000.0)
        debt[0] = 0.0
        if b + 1 < B:
            push_qkv_items(b + 1)
            box = []
            fillq.append((0.0, lambda box=box: box.append(1)))
            qkv_done[b + 1] = box
        ot_sb = otpool.tile([128, HPC, T], dt.bfloat16, tag="ot")
        qT_sb, kT_sb, v_sb = qkv_tiles.pop(b)
        budget = 850.0 if b + 1 < B else 400.0
        pending_norm = None
        for c in range(T // 512):          # qt chunks
            for h in range(HPC):
                ot_ps = ps_ot.tile([128, 512], dt.float32, tag="acc")
                qs = qT_sb[:, h, c * 512:(c + 1) * 512]
                pts = []
                nodes = {}

                def tree_add(spec):
                    name, ia, ib = spec
                    a = pts[ia] if isinstance(ia, int) else nodes[ia]
                    bb = pts[ib] if isinstance(ib, int) else nodes[ib]
                    s = sums.tile([128, 512], dt.bfloat16, tag="sum",
                                  name="psum_tree")
                    nc.vector.tensor_add(out=s, in0=a, in1=bb)
                    nodes[name] = s

                for kt in range(T // 128):  # 16 key tiles
                    st_ps = ps_st.tile([128, 512], dt.float32, tag="st")
                    nc.tensor.matmul(
                        st_ps, lhsT=kT_sb[:, h, kt * 128:(kt + 1) * 128],
                        rhs=qs, start=True, stop=True)
                    pt = ptpool.tile([128, 512], dt.bfloat16, tag="pt")
                    nc.scalar.activation(out=pt, in_=st_ps, func=AF.Exp,
                                         scale=SCALE)
                    pts.append(pt)
                    if kt >= 2:  # PV two steps behind ST (hide exp latency)
                        nc.tensor.matmul(
                            ot_ps, lhsT=v_sb[:, kt - 2, h * 128:(h + 1) * 128],
                            rhs=pts[kt - 2], start=(kt == 2), stop=False,
                            skip_group_check=True)
                    if kt == 3 and pending_norm is not None:
                        emit_norm(pending_norm)
                        pending_norm = None
                    if "norowsum" not in ablate:
                        for spec in TREE_AT.get(kt, ()):
                            tree_add(spec)
                    if "nointerleave" not in ablate:
                        drain(budget)
                if "nointerleave" not in ablate:
                    drain(budget)
                nc.tensor.matmul(
                    ot_ps, lhsT=v_sb[:, 14, h * 128:(h + 1) * 128],
                    rhs=pts[14], start=False, stop=False,
                    skip_group_check=True)
                if "nointerleave" not in ablate:
                    drain(budget)
                nc.tensor.matmul(
                    ot_ps, lhsT=v_sb[:, 15, h * 128:(h + 1) * 128],
                    rhs=pts[15], start=False, stop=True,
                    skip_group_check=True)
                dst = ot_sb[:, h, c * 512:(c + 1) * 512]
                if "norowsum" in ablate:
                    nc.vector.tensor_copy(out=dst, in_=ot_ps)
                    if h == 1:
                        push_proj_items(b, c, ot_sb)
                else:
                    for spec in TREE_TAIL:
                        tree_add(spec)
                    pending_norm = (nodes["tot"], ot_ps, dst, b, c, h,
                                    ot_sb)
        if pending_norm is not None:  # last unit of the batch
            emit_norm(pending_norm)
            pending_norm = None
    drain_all()


def prep_inputs(x: np.ndarray, w1: np.ndarray, w2: np.ndarray):
    """Host-side shard + transpose + bf16 cast. Returns in_maps for 8 cores."""
    # pretile x: chunk ci of 512 tokens -> [128 part, 16 e-tiles, 512 tok]
    xb = x.reshape(N // 512, 512, ET, 128).transpose(0, 3, 2, 1)
    xf = np.ascontiguousarray(xb).astype(BF16)
    w1r = w1.reshape(3, NH, D, E)
    in_maps = []
    for m in range(NCORES):
        hs = slice(HPC * m, HPC * (m + 1))
        wq = np.ascontiguousarray(w1r[0, hs].reshape(F, E).T).astype(BF16)
        wk = np.ascontiguousarray(w1r[1, hs].reshape(F, E).T).astype(BF16)
        wv = np.ascontiguousarray(w1r[2, hs].reshape(F, E).T).astype(BF16)
        w2s = np.ascontiguousarray(w2[:, F * m:F * (m + 1)].T).astype(BF16)
        in_maps.append({"xT": xf, "wqT": wq, "wkT": wk, "wvT": wv, "w2sT": w2s})
    return in_maps


def run(x, w1, w2, trace=False):
    from concourse import bass_utils

    nc = build_nc()
    in_maps = prep_inputs(np.asarray(x), np.asarray(w1), np.asarray(w2))
    res = bass_utils.run_bass_kernel_spmd(
        nc, in_maps, core_ids=list(range(NCORES)), trace=trace)
    acc = np.zeros((N, E), np.float32)
    for r in res.results:
        acc += r["out"]
    return acc.reshape(B, T, E), res


def kernel(x, w1, w2):
    out, _ = run(x, w1, w2, trace=False)
    return out


# revision 32
# speedup vs baseline: 1.0417x; 1.0417x over previous
"""Tensor-parallel fused attention block (QKV proj + MHA + out proj) for 8 TRN2 cores.

Sharding: 16 heads -> 2 heads per core. W1 rows (q/k/v of the core's heads) and
W2 columns are sharded; x is replicated. Each core computes a partial output
[B*T, E] (its heads' contribution through W2); the host sums the 8 partials.

v3: single software-pipelined PE stream. The attention inner loop (ST -> exp ->
PV, with PV emitted two kt behind ST so the ACT exp latency is always hidden)
is the backbone; QKV matmul groups of the NEXT batch and out-projection groups
of finished chunks are interleaved into it as cost-budgeted filler so the
in-order PE queue never idles while ACT paces exp. Softmax denominators per
(head, chunk): a DVE pairwise tree sums the 16 exp tiles (rowsum matmuls
measured serial on HW - 16 matmuls/unit cost 169 us/iter in the v2 design),
then ONE ones_col[128,128] bf16 matmul both reduces the summed tile over
partitions and broadcasts the denominator, then DVE reciprocal + multiply.
ST pairs write one [128,1024] two-bank PSUM tile so a single ACT exp covers
two kt tiles ((224+1024)/1.2 vs 2x(224+512)/1.2 cycles - 15% less ACT busy
and half the activation instructions); PV runs three kt behind ST.
That norm chain is deferred into the NEXT unit's early kt slots so the DVE
tail never stalls PE; it is emitted inline (never queued behind filler, which
would deadlock single-buffered PSUM pools against the in-order engine queues).
Batch boundaries force-drain only the next batch's QKV items (correctness:
their writes must be emitted before the attention reads / pool-slot reuse);
leftover projection filler flows into the next batch.

Layouts (host-prepped, all bf16):
  xT   [E, B*T]   x transposed, feature-major (rhs/lhsT tiles for QKV matmuls)
  wqT  [E, 256]   w1-q rows for the core's 2 heads, transposed
  wkT  [E, 256]
  wvT  [E, 256]
  w2sT [256, E]   w2 columns for the core's heads, transposed
"""

import collections
import json
import types

import ml_dtypes
import numpy as np

B, T, E = 4, 2048, 2048
NH, D = 16, 128
NCORES = 8
HPC = NH // NCORES          # heads per core = 2
F = HPC * D                 # per-core qkv feature dim = 256
N = B * T                   # 8192 tokens
ET = E // 128               # 16 e-tiles
SCALE = float(1.0 / np.sqrt(D))

BF16 = ml_dtypes.bfloat16
_BUDGETS = [700.0, 500.0]
_RESERVE = [40]


def _split_multi_waits(m: dict) -> dict:
    """This container's walrus rejects any instruction carrying >1 semaphore
    wait; hoist extra waits into standalone single-wait EventSemaphore insts
    emitted just before, on the same engine (program order preserves semantics)."""
    for fn in m["functions"]:
        for b in fn["blocks"]:
            new_insts = []
            for i in b["instructions"]:
                si = i.get("sync_info")
                waits = (si or {}).get("on_wait") or []
                if len(waits) > 1:
                    for k, w in enumerate(waits[:-1]):
                        new_insts.append({
                            "name": f"{i['name']}-presplitwait-{k}",
                            "opcode": "EventSemaphore",
                            "engine": i["engine"],
                            "ins": [], "outs": [],
                            "sync_info": {"on_wait": [w], "on_update": []},
                        })
                    si["on_wait"] = [waits[-1]]
                new_insts.append(i)
            b["instructions"] = new_insts
    return m


def _patch_serializer(nc):
    orig = nc.to_json_bytes

    def to_json_bytes(self):
        return json.dumps(_split_multi_waits(json.loads(orig()))).encode()

    nc.to_json_bytes = types.MethodType(to_json_bytes, nc)


def build_nc(loop: int | None = None, ablate: frozenset | set = frozenset()):
    import contextlib

    import concourse.bass as bass
    import concourse.mybir as mybir
    import concourse.tile as tile

    dt = mybir.dt
    AF = mybir.ActivationFunctionType

    nc = bass.Bass("TRN2", target_bir_lowering=False, debug=False)

    xT = nc.dram_tensor("xT", [N // 512, 128, ET, 512], dt.bfloat16,
                        kind="ExternalInput")
    wqT = nc.dram_tensor("wqT", [E, F], dt.bfloat16, kind="ExternalInput")
    wkT = nc.dram_tensor("wkT", [E, F], dt.bfloat16, kind="ExternalInput")
    wvT = nc.dram_tensor("wvT", [E, F], dt.bfloat16, kind="ExternalInput")
    w2sT = nc.dram_tensor("w2sT", [F, E], dt.bfloat16, kind="ExternalInput")
    out = nc.dram_tensor("out", [N, E], dt.bfloat16, kind="ExternalOutput")

    wq_r = wqT.rearrange("(n p) f -> p n f", p=128)     # [128, 16, 256]
    wk_r = wkT.rearrange("(n p) f -> p n f", p=128)
    wv_r = wvT.rearrange("(n p) f -> p n f", p=128)
    w2_r = w2sT.rearrange("(n p) f -> p n f", p=128)    # [128, 2, 2048]

    with tile.TileContext(nc) as tc:
        with (
            tc.tile_pool(name="wpool", bufs=1) as wpool,
            tc.tile_pool(name="xpool", bufs=3) as xpool,
            tc.tile_pool(name="qkv", bufs=2) as qkvpool,
            tc.tile_pool(name="ptpool", bufs=10) as ptpool,
            tc.tile_pool(name="otpool", bufs=2) as otpool,
            tc.tile_pool(name="opool", bufs=3) as opool,
            tc.tile_pool(name="small", bufs=2) as small,
            tc.tile_pool(name="sums", bufs=6) as sums,
            tc.tile_pool(name="ps_mm", bufs=2, space="PSUM") as ps_mm,
            tc.tile_pool(name="ps_st", bufs=2, space="PSUM") as ps_st,
            tc.tile_pool(name="ps_ot", bufs=2, space="PSUM") as ps_ot,
        ):
            # weights + constants (loaded once)
            wq_sb = wpool.tile([128, ET, F], dt.bfloat16)
            wk_sb = wpool.tile([128, ET, F], dt.bfloat16)
            wv_sb = wpool.tile([128, ET, F], dt.bfloat16)
            w2_sb = wpool.tile([128, 2, E], dt.bfloat16)
            # wq first: the opening q-projection groups wait only on wq +
            # x chunk 0, so those head every DMA queue; wk/wv/w2 follow
            for g in range(4):
                sl = slice(g * 4, (g + 1) * 4)
                nc.sync.dma_start(out=wq_sb[:, sl, :], in_=wq_r[:, sl, :])
            for g in range(4):
                sl = slice(g * 4, (g + 1) * 4)
                nc.sync.dma_start(out=wk_sb[:, sl, :], in_=wk_r[:, sl, :])
            for g in range(4):
                sl = slice(g * 4, (g + 1) * 4)
                nc.sync.dma_start(out=wv_sb[:, sl, :], in_=wv_r[:, sl, :])
                nc.sync.dma_start(
                    out=w2_sb[:, :, g * 512:(g + 1) * 512],
                    in_=w2_r[:, :, g * 512:(g + 1) * 512])
            # all-ones [128,128]: one bf16 matmul both reduces the partition
            # dim and broadcasts the total to every output partition
            ones_col = wpool.tile([128, 128], dt.bfloat16)
            nc.vector.memset(ones_col, 1.0)

            loop_cm = tc.For_i(0, loop, 1) if loop else contextlib.nullcontext()
            with loop_cm:
                _emit_body(nc, tc, dt, AF, locals(), ablate)
    _patch_serializer(nc)
    return nc


def _emit_body(nc, tc, dt, AF, env, ablate=frozenset()):
    xT_r, out = env["xT"], env["out"]
    wq_sb, wk_sb, wv_sb, w2_sb = env["wq_sb"], env["wk_sb"], env["wv_sb"], env["w2_sb"]
    ones_col = env["ones_col"]
    xpool, qkvpool, ptpool, otpool, opool, small, sums = (
        env["xpool"], env["qkvpool"], env["ptpool"], env["otpool"],
        env["opool"], env["small"], env["sums"])
    ps_mm, ps_st, ps_ot = env["ps_mm"], env["ps_st"], env["ps_ot"]

    # filler queues of (pe_cost_ns, emit_fn); drained on a cost budget so the
    # chunky items spread evenly across the attention kt slots. proj items are
    # hoarded (RESERVE) during batches 0..B-2 so the last batch - which has no
    # next-batch QKV to interleave - still has PE filler for its exp-paced
    # stretches. proj(b) only has to finish before batch b+2's norm writes
    # reuse its ot_sb pool slot, so one-batch deferral is dependency-safe.
    fillq = collections.deque()   # qkv items (+ sentinels)
    projq = collections.deque()   # out-projection items
    debt = [0.0]
    RESERVE = _RESERVE[0]

    def drain(budget, reserve=0):
        debt[0] += budget
        while debt[0] > 0:
            if fillq:
                cost, fn = fillq.popleft()
            elif len(projq) > reserve:
                cost, fn = projq.popleft()
            else:
                break
            fn()
            debt[0] -= cost

    def drain_all():
        while fillq:
            fillq.popleft()[1]()
        while projq:
            projq.popleft()[1]()
        debt[0] = 0.0

    qkv_tiles = {}

    def push_qkv_items(b):
        """Queue batch b's QKV projection as whole-group filler items (each
        item allocates, fills, and evacuates its own PSUM bank - holding a
        pool slot across items would deadlock the in-order PE queue)."""
        qkv_tiles[b] = (
            qkvpool.tile([128, HPC, T], dt.bfloat16, tag="qT", name="qT_sb"),
            qkvpool.tile([128, HPC, T], dt.bfloat16, tag="kT", name="kT_sb"),
            qkvpool.tile([128, T // 128, F], dt.bfloat16, tag="v", name="v_sb"),
        )
        qT_sb, kT_sb, v_sb = qkv_tiles[b]
        for c in range(T // 512):  # 4 chunks of 512 tokens
            x_tile = xpool.tile([128, ET, 512], dt.bfloat16, tag="x")
            ci = b * 4 + c

            def dma_x(x_tile=x_tile, ci=ci):
                for g in range(4):
                    nc.sync.dma_start(
                        out=x_tile[:, g * 4:(g + 1) * 4, :],
                        in_=xT_r[ci, :, g * 4:(g + 1) * 4, :])
            fillq.append((0.0, dma_x))

            for wi, (w_sb, dst) in enumerate(
                    ((wq_sb, qT_sb), (wk_sb, kT_sb))):
                for h in range(HPC):
                    def qk_grp(w_sb=w_sb, dst=dst, x_tile=x_tile, h=h, c=c,
                               wi=wi):
                        ps = ps_mm.tile([128, 512], dt.float32, tag="mm")
                        for e in range(ET):
                            nc.tensor.matmul(
                                ps, lhsT=w_sb[:, e, h * 128:(h + 1) * 128],
                                rhs=x_tile[:, e, :],
                                start=(e == 0), stop=(e == ET - 1))
                        nc.vector.tensor_copy(
                            out=dst[:, h, c * 512:(c + 1) * 512], in_=ps)
                    fillq.append((3413.0, qk_grp))

            for nn in range(4):  # token tiles of 128 within chunk
                def v_grp(x_tile=x_tile, nn=nn, c=c, v_sb=v_sb):
                    ps = ps_mm.tile([128, F], dt.float32, tag="mm")
                    for e in range(ET):
                        nc.tensor.matmul(
                            ps, lhsT=x_tile[:, e, nn * 128:(nn + 1) * 128],
                            rhs=wv_sb[:, e, :],
                            start=(e == 0), stop=(e == ET - 1))
                    nc.vector.tensor_copy(out=v_sb[:, c * 4 + nn, :],
                                          in_=ps)
                fillq.append((1707.0, v_grp))

    def push_proj_items(b, c, ot_sb):
        """Out-projection for token chunk c of batch b (both heads normalized)."""
        for nn in range(c * 4, (c + 1) * 4):  # 4 token tiles of 128
            o_sb = opool.tile([128, E], dt.bfloat16, tag="o")
            for oc in range(E // 512):
                def p_grp(ot_sb=ot_sb, o_sb=o_sb, nn=nn, oc=oc):
                    ps = ps_mm.tile([128, 512], dt.float32, tag="mm")
                    for j in range(HPC):
                        nc.tensor.matmul(
                            ps, lhsT=ot_sb[:, j, nn * 128:(nn + 1) * 128],
                            rhs=w2_sb[:, j, oc * 512:(oc + 1) * 512],
                            start=(j == 0), stop=(j == HPC - 1))
                    cp = (nc.vector.tensor_copy if oc % 2 == 0
                          else nc.scalar.copy)
                    cp(out=o_sb[:, oc * 512:(oc + 1) * 512], in_=ps)
                projq.append((480.0, p_grp))

            def p_dma(o_sb=o_sb, b=b, nn=nn):
                nc.sync.dma_start(
                    out=out[b * T + nn * 128:b * T + (nn + 1) * 128, :],
                    in_=o_sb)
            projq.append((0.0, p_dma))

    # ---- batch 0 QKV: emitted dense upfront ----
    push_qkv_items(0)
    drain_all()

    def emit_norm(pn):
        """Deferred normalization for a finished unit: one ones_col matmul
        reduces the summed-P tile over partitions AND broadcasts the
        denominator; then DVE reciprocal + multiply. Emitted during the NEXT
        unit's early kt slots so the DVE tree-add tail never stalls PE."""
        total, ot_ps, dst, bb, cc, hh, ot_sb_ = pn
        bc2 = ps_st.tile([128, 1024], dt.float32, tag="st", name="bc2")
        bc = bc2[:, 0:512]
        nc.tensor.matmul(bc, lhsT=ones_col, rhs=total, start=True, stop=True)
        recip = small.tile([128, 512], dt.float32, tag="recip")
        nc.vector.reciprocal(out=recip, in_=bc)
        nc.vector.tensor_mul(out=dst, in0=ot_ps, in1=recip)
        if hh == 1:
            push_proj_items(bb, cc, ot_sb_)

    # DVE pairwise tree over the 8 double-width exp tiles (pair p covers kt
    # 2p,2p+1); emit each add once both operands' exps are comfortably old
    TREE_AT = {5: [("a0", 0, 1)], 9: [("a1", 2, 3)],
               10: [("b0", "a0", "a1")], 13: [("a2", 4, 5)]}
    TREE_TAIL = [("a3", 6, 7), ("b1", "a2", "a3"), ("c", "b0", "b1")]

    qkv_done = {}
    for b in range(B):
        # correctness barrier: batch b's attention reads qkv(b) tiles, whose
        # writer items must be EMITTED before the first ST reads them - and
        # before push_qkv_items(b+1) reuses their x-tile pool slots
        while b in qkv_done and not qkv_done[b]:
            drain(100000.0, reserve=10**9)  # qkv only - keep the proj hoard
        debt[0] = 0.0
        if b + 1 < B:
            push_qkv_items(b + 1)
            box = []
            fillq.append((0.0, lambda box=box: box.append(1)))
            qkv_done[b + 1] = box
        ot_sb = otpool.tile([128, HPC, T], dt.bfloat16, tag="ot")
        qT_sb, kT_sb, v_sb = qkv_tiles.pop(b)
        budget = _BUDGETS[0] if b + 1 < B else _BUDGETS[1]
        reserve = RESERVE if b + 1 < B else 0
        pending_norm = None
        for c in range(T // 512):          # qt chunks
            for h in range(HPC):
                ot_ps = ps_ot.tile([128, 512], dt.float32, tag="acc")
                qs = qT_sb[:, h, c * 512:(c + 1) * 512]
                pairs = []      # [128,1024] exp tiles, pair p = kt 2p,2p+1
                nodes = {}

                def pt_half(kt):
                    return pairs[kt // 2][:, (kt % 2) * 512:
                                          (kt % 2) * 512 + 512]

                def tree_add(spec):
                    name, ia, ib = spec
                    a = pairs[ia] if isinstance(ia, int) else nodes[ia]
                    bb = pairs[ib] if isinstance(ib, int) else nodes[ib]
                    s = sums.tile([128, 1024], dt.bfloat16, tag="sum",
                                  name="psum_tree")
                    nc.vector.tensor_add(out=s, in0=a, in1=bb)
                    nodes[name] = s

                st2 = None
                for kt in range(T // 128):  # 16 key tiles, as 8 pairs
                    if kt % 2 == 0:
                        st2 = ps_st.tile([128, 1024], dt.float32, tag="st")
                    nc.tensor.matmul(
                        st2[:, (kt % 2) * 512:(kt % 2) * 512 + 512],
                        lhsT=kT_sb[:, h, kt * 128:(kt + 1) * 128],
                        rhs=qs, start=True, stop=True)
                    if kt % 2 == 1:  # one exp covers the pair (2 PSUM banks)
                        pt2 = ptpool.tile([128, 1024], dt.bfloat16, tag="pt")
                        nc.scalar.activation(out=pt2, in_=st2, func=AF.Exp,
                                             scale=SCALE)
                        pairs.append(pt2)
                    if kt >= 3:  # PV three steps behind ST (pair exp latency)
                        nc.tensor.matmul(
                            ot_ps, lhsT=v_sb[:, kt - 3, h * 128:(h + 1) * 128],
                            rhs=pt_half(kt - 3), start=(kt == 3), stop=False,
                            skip_group_check=True)
                    if kt == 3 and pending_norm is not None:
                        emit_norm(pending_norm)
                        pending_norm = None
                    if "norowsum" not in ablate:
                        for spec in TREE_AT.get(kt, ()):
                            tree_add(spec)
                    if "nointerleave" not in ablate:
                        drain(budget, reserve)
                for kt in (13, 14, 15):
                    if "nointerleave" not in ablate:
                        drain(budget, reserve)
                    nc.tensor.matmul(
                        ot_ps, lhsT=v_sb[:, kt, h * 128:(h + 1) * 128],
                        rhs=pt_half(kt), start=False, stop=(kt == 15),
                        skip_group_check=True)
                dst = ot_sb[:, h, c * 512:(c + 1) * 512]
                if "norowsum" in ablate:
                    nc.vector.tensor_copy(out=dst, in_=ot_ps)
                    if h == 1:
                        push_proj_items(b, c, ot_sb)
                else:
                    for spec in TREE_TAIL:
                        tree_add(spec)
                    tot = sums.tile([128, 512], dt.bfloat16, tag="sum",
                                    name="psum_tot")
                    nc.vector.tensor_add(out=tot, in0=nodes["c"][:, 0:512],
                                         in1=nodes["c"][:, 512:1024])
                    pending_norm = (tot, ot_ps, dst, b, c, h, ot_sb)
        if pending_norm is not None:  # last unit of the batch
            emit_norm(pending_norm)
            pending_norm = None
    drain_all()


def prep_inputs(x: np.ndarray, w1: np.ndarray, w2: np.ndarray):
    """Host-side shard + transpose + bf16 cast. Returns in_maps for 8 cores."""
    # pretile x: chunk ci of 512 tokens -> [128 part, 16 e-tiles, 512 tok]
    xb = x.reshape(N // 512, 512, ET, 128).transpose(0, 3, 2, 1)
    xf = np.ascontiguousarray(xb).astype(BF16)
    w1r = w1.reshape(3, NH, D, E)
    in_maps = []
    for m in range(NCORES):
        hs = slice(HPC * m, HPC * (m + 1))
        wq = np.ascontiguousarray(w1r[0, hs].reshape(F, E).T).astype(BF16)
        wk = np.ascontiguousarray(w1r[1, hs].reshape(F, E).T).astype(BF16)
        wv = np.ascontiguousarray(w1r[2, hs].reshape(F, E).T).astype(BF16)
        w2s = np.ascontiguousarray(w2[:, F * m:F * (m + 1)].T).astype(BF16)
        in_maps.append({"xT": xf, "wqT": wq, "wkT": wk, "wvT": wv, "w2sT": w2s})
    return in_maps


def run(x, w1, w2, trace=False):
    from concourse import bass_utils

    nc = build_nc()
    in_maps = prep_inputs(np.asarray(x), np.asarray(w1), np.asarray(w2))
    res = bass_utils.run_bass_kernel_spmd(
        nc, in_maps, core_ids=list(range(NCORES)), trace=trace)
    acc = np.zeros((N, E), np.float32)
    for r in res.results:
        acc += r["out"]
    return acc.reshape(B, T, E), res


def kernel(x, w1, w2):
    out, _ = run(x, w1, w2, trace=False)
    return out
